# revision 20
# baseline (speedup 1.0000x reference)
"""CrossAttention2d Trainium2 kernel.

Strategy: data-parallel over batch N=16 across 8 NeuronCores (2 samples per
core), no collectives. All projections run as bf16 matmuls with fp32 PSUM
accumulation. Host-side preprocessing folds:
  - LayerNorm affine (ln_w, ln_b) into kv_w / kv_b
  - attention scale d^-0.25 into q_w/q_b and the K half of kv_w/kv_b
  - out_b into the V bias via delta = solve(out_w, out_b) (softmax rows sum
    to 1, so adding delta to v adds out_w@delta = out_b to the output)
  - weight transposes to [cin, cout] (lhsT layout), chunked to [128, kc, cout]

Per-sample on-chip pipeline:
  GroupNorm stats (DVE reduce + fused square-reduce, partition-sum via ones
  matmul) -> AdaGN scale/shift via matmul -> per-channel affine -> q proj
  -> encoder LN -> DMA-transpose -> kv proj -> per-head attention
  (att^T = k_h^T q_h in PSUM, Exp with mask bias on ACT, y^T = v_aug^T att
  with a ones column producing the softmax denominator) -> batched
  reciprocal -> DMA partition-broadcast -> normalize -> out proj -> residual.
"""

import numpy as np
import ml_dtypes

import concourse.bass as bass
import concourse.mybir as mybir
import concourse.tile as tile
from concourse import bacc
from concourse.bass import ts
from concourse.bass_utils import run_bass_kernel_spmd

F32 = mybir.dt.float32
BF16 = mybir.dt.bfloat16
AX = mybir.AxisListType
ALU = mybir.AluOpType
ACTF = mybir.ActivationFunctionType

N_CORES = 8
N, C, H, W = 16, 512, 32, 32
HW = H * W                     # 1024
CE, S, NH = 768, 77, 8
D = C // NH                    # 64
NS = N // N_CORES              # 2 samples per core
CDC = C // 128                 # 4 chunks of c_dec
CEC = CE // 128                # 6 chunks of c_enc
EPS = 1e-5
EL = C * HW                    # groupnorm element count
SCALE = float(D) ** (-0.25)


def build_program(reps: int = 1):
    nc = bacc.Bacc("TRN2", target_bir_lowering=False, debug=False,
                   num_devices=N_CORES)

    # ---- DRAM I/O (per core) ----
    x_d = nc.dram_tensor("x", [NS, C, HW], F32, kind="ExternalInput")
    enc_d = nc.dram_tensor("enc", [NS, S, CE], F32, kind="ExternalInput")
    condT_d = nc.dram_tensor("condT", [128, CDC, NS], BF16, kind="ExternalInput")
    maskb_d = nc.dram_tensor("maskb", [S, NS], F32, kind="ExternalInput")
    adagn_wT_d = nc.dram_tensor("adagn_wT", [128, CDC, 2 * C], BF16, kind="ExternalInput")
    adagn_b_d = nc.dram_tensor("adagn_b", [128, 2 * CDC], F32, kind="ExternalInput")
    q_wT_d = nc.dram_tensor("q_wT", [128, CDC, C], BF16, kind="ExternalInput")
    q_b_d = nc.dram_tensor("q_b", [128, CDC], F32, kind="ExternalInput")
    kv_wT_d = nc.dram_tensor("kv_wT", [128, CEC, 2 * C], BF16, kind="ExternalInput")
    kv_b_k_d = nc.dram_tensor("kv_b_k", [128, CDC], F32, kind="ExternalInput")
    kv_b_v_d = nc.dram_tensor("kv_b_v", [1, C], BF16, kind="ExternalInput")
    out_wT_d = nc.dram_tensor("out_wT", [128, CDC, C], BF16, kind="ExternalInput")
    out_d = nc.dram_tensor("out", [NS, C, HW], F32, kind="ExternalOutput")

    with tile.TileContext(nc) as tc:
        import contextlib
        with contextlib.ExitStack() as ctx:
            wp = ctx.enter_context(tc.tile_pool(name="weights", bufs=1))
            xp_pool = ctx.enter_context(tc.tile_pool(name="xtiles", bufs=2))
            sp = ctx.enter_context(tc.tile_pool(name="small", bufs=2))
            scratchp = ctx.enter_context(tc.tile_pool(name="scratch", bufs=1))
            attp = ctx.enter_context(tc.tile_pool(name="attsb", bufs=3))
            y65p = ctx.enter_context(tc.tile_pool(name="y65", bufs=8))
            outp = ctx.enter_context(tc.tile_pool(name="outsb", bufs=2))
            psA = ctx.enter_context(tc.tile_pool(name="psA", bufs=2, space="PSUM"))
            psY = ctx.enter_context(tc.tile_pool(name="psY", bufs=1, space="PSUM"))
            psS = ctx.enter_context(tc.tile_pool(name="psS", bufs=2, space="PSUM"))
            dramp = ctx.enter_context(tc.tile_pool(name="dram", bufs=2, space="DRAM"))

            def body():
                # ---------- load weights / constants ----------
                adagn_wT = wp.tile([128, CDC, 2 * C], BF16)
                nc.sync.dma_start(adagn_wT[:], adagn_wT_d[:])
                q_wT = wp.tile([128, CDC, C], BF16)
                nc.sync.dma_start(q_wT[:], q_wT_d[:])
                kv_wT = wp.tile([128, CEC, 2 * C], BF16)
                nc.sync.dma_start(kv_wT[:], kv_wT_d[:])
                out_wT = wp.tile([128, CDC, C], BF16)
                nc.sync.dma_start(out_wT[:], out_wT_d[:])
                adagn_b = wp.tile([128, 2 * CDC], F32)
                nc.sync.dma_start(adagn_b[:], adagn_b_d[:])
                q_b = wp.tile([128, CDC], F32)
                nc.sync.dma_start(q_b[:], q_b_d[:])
                kv_b_k = wp.tile([128, CDC], F32)
                nc.sync.dma_start(kv_b_k[:], kv_b_k_d[:])
                kv_b_v = wp.tile([1, C], BF16)
                nc.sync.dma_start(kv_b_v[:], kv_b_v_d[:])
                condT = wp.tile([128, CDC, NS], BF16)
                nc.sync.dma_start(condT[:], condT_d[:])
                maskb = wp.tile([S, NS], F32)
                nc.sync.dma_start(maskb[:], maskb_d[:])

                ones128 = wp.tile([128, 1], F32)
                nc.vector.memset(ones128[:], 1.0)
                ones1w = wp.tile([1, 128], F32)
                nc.vector.memset(ones1w[:], 1.0)
                ones77 = wp.tile([1, S], BF16)
                nc.vector.memset(ones77[:], 1.0)
                ident = wp.tile([128, 128], BF16)
                from concourse.masks import make_identity
                make_identity(nc, ident[:])
                eps_t = wp.tile([128, 1], F32)
                nc.vector.memset(eps_t[:], EPS)

                # ---------- AdaGN scale/shift for both samples ----------
                # ss[o, n] = sum_f adagn_wT[f, o] * condT[f, n]
                ss_ps = psS.tile([128, 2 * CDC, NS], F32, tag="sm")
                for oc in range(2 * CDC):
                    for kc in range(CDC):
                        nc.tensor.matmul(
                            ss_ps[:, oc, :],
                            adagn_wT[:, kc, ts(oc, 128)],
                            condT[:, kc, :],
                            start=(kc == 0), stop=(kc == CDC - 1),
                        )
                ss_sb = wp.tile([128, 2 * CDC, NS], F32)
                nc.vector.tensor_copy(ss_sb[:], ss_ps[:])

                eT = wp.tile([128, CEC, NS * 80], BF16)
                for n in range(NS):
                    sample(n, adagn_wT, q_wT, kv_wT, out_wT, adagn_b, q_b,
                           kv_b_k, kv_b_v, maskb, ones128, ones1w, ones77,
                           ss_sb, eT, ident, eps_t)

            def sample(n, adagn_wT, q_wT, kv_wT, out_wT, adagn_b, q_b,
                       kv_b_k, kv_b_v, maskb, ones128, ones1w, ones77,
                       ss_sb, eT, ident, eps_t):
                # ---------- load x ----------
                x_t = xp_pool.tile([128, CDC, HW], F32, tag="x")
                nc.sync.dma_start(
                    x_t[:], x_d[n].rearrange("(c p) w -> p c w", p=128))

                # ---------- GroupNorm stats ----------
                partials = sp.tile([128, 2], F32, tag="partials")
                nc.vector.tensor_reduce(
                    partials[:, 0:1], x_t[:], AX.XY, ALU.add)
                sq_scr = scratchp.tile([128, CDC, HW], BF16, tag="sqscr")
                nc.scalar.activation(sq_scr[:], x_t[:], ACTF.Square,
                                     accum_out=partials[:, 1:2])
                stats_ps = psS.tile([1, 2], F32, tag="sm")
                nc.tensor.matmul(stats_ps[:], ones128[:], partials[:])
                # mu, rs on partition 0
                stat_s = sp.tile([1, 2], F32, tag="stat_s")
                tmp4 = sp.tile([1, 3], F32, tag="tmp4")
                nc.vector.tensor_scalar_mul(stat_s[:, 0:1], stats_ps[:, 0:1], 1.0 / EL)
                nc.vector.tensor_scalar_mul(tmp4[:, 0:1], stats_ps[:, 1:2], 1.0 / EL)
                nc.vector.tensor_mul(tmp4[:, 1:2], stat_s[:, 0:1], stat_s[:, 0:1])
                nc.vector.tensor_sub(tmp4[:, 2:3], tmp4[:, 0:1], tmp4[:, 1:2])
                std1 = sp.tile([1, 1], F32, tag="std1")
                nc.scalar.activation(std1[:], tmp4[:, 2:3], ACTF.Sqrt,
                                     bias=eps_t[0:1, :])
                nc.vector.reciprocal(stat_s[:, 1:2], std1[:])
                bc_ps = psS.tile([128, 2], F32, tag="sm")
                nc.tensor.matmul(bc_ps[:], ones1w[:], stat_s[:])
                mu_c, rs_c = bc_ps[:, 0:1], bc_ps[:, 1:2]

                # ---------- AdaGN a/b coefficients ----------
                a_n = sp.tile([128, CDC], F32, tag="a_n")
                b_n = sp.tile([128, CDC], F32, tag="b_n")
                t_sc = sp.tile([128, CDC], F32, tag="t_sc")
                t_sh = sp.tile([128, CDC], F32, tag="t_sh")
                nc.vector.tensor_add(t_sc[:], ss_sb[:, 0:CDC, n], adagn_b[:, 0:CDC])
                nc.vector.tensor_scalar(
                    a_n[:], t_sc[:], scalar1=rs_c, scalar2=rs_c,
                    op0=ALU.mult, op1=ALU.add)
                nc.vector.tensor_add(t_sh[:], ss_sb[:, CDC:2 * CDC, n],
                                     adagn_b[:, CDC:2 * CDC])
                t_amu = sp.tile([128, CDC], F32, tag="t_amu")
                nc.vector.tensor_scalar(
                    t_amu[:], a_n[:], scalar1=mu_c, scalar2=None, op0=ALU.mult)
                nc.vector.tensor_sub(b_n[:], t_sh[:], t_amu[:])

                # ---------- apply AdaGN -> x' (bf16) ----------
                xp_t = xp_pool.tile([128, CDC, HW], BF16, tag="xp")
                for c in range(CDC):
                    nc.vector.tensor_scalar(
                        xp_t[:, c, :], x_t[:, c, :],
                        scalar1=a_n[:, c:c + 1], scalar2=b_n[:, c:c + 1],
                        op0=ALU.mult, op1=ALU.add)

                # ---------- q projection ----------
                q_bf = xp_pool.tile([128, CDC, HW], BF16, tag="qbf")
                for oc in range(CDC):
                    q_ps = psA.tile([128, HW], F32, tag="big2")
                    for kc in range(CDC):
                        for nh2 in range(2):
                            nc.tensor.matmul(
                                q_ps[:, ts(nh2, 512)],
                                q_wT[:, kc, ts(oc, 128)],
                                xp_t[:, kc, ts(nh2, 512)],
                                start=(kc == 0), stop=(kc == CDC - 1))
                    nc.scalar.activation(q_bf[:, oc, :], q_ps[:],
                                         ACTF.Identity, bias=q_b[:, oc:oc + 1])

                # ---------- encoder LN ----------
                e_raw = sp.tile([S, CE], F32, tag="e_raw")
                nc.sync.dma_start(e_raw[:], enc_d[n])
                esum = sp.tile([S, 2], F32, tag="esum")
                nc.vector.tensor_reduce(esum[:, 0:1], e_raw[:], AX.X, ALU.add)
                esq_scr = scratchp.tile([S, CE], BF16, tag="esqscr")
                nc.scalar.activation(esq_scr[:], e_raw[:], ACTF.Square,
                                     accum_out=esum[:, 1:2])
                emu = sp.tile([S, 4], F32, tag="emu")
                nc.vector.tensor_scalar_mul(emu[:, 0:1], esum[:, 0:1], 1.0 / CE)
                nc.vector.tensor_scalar_mul(emu[:, 1:2], esum[:, 1:2], 1.0 / CE)
                nc.vector.tensor_mul(emu[:, 2:3], emu[:, 0:1], emu[:, 0:1])
                nc.vector.tensor_sub(emu[:, 3:4], emu[:, 1:2], emu[:, 2:3])
                estd = sp.tile([S, 2], F32, tag="estd")
                nc.scalar.activation(estd[:, 0:1], emu[:, 3:4], ACTF.Sqrt,
                                     bias=eps_t[0:S, :])
                nc.vector.reciprocal(estd[:, 1:2], estd[:, 0:1])
                negmurs = sp.tile([S, 1], F32, tag="negmurs")
                nc.vector.tensor_scalar(
                    negmurs[:], emu[:, 0:1], scalar1=estd[:, 1:2], scalar2=-1.0,
                    op0=ALU.mult, op1=ALU.mult)
                xn = sp.tile([80, CE], BF16, tag="xn")
                nc.vector.memset(xn[:], 0.0)
                nc.scalar.activation(xn[0:S, :], e_raw[:], ACTF.Identity,
                                     bias=negmurs[:], scale=estd[:, 1:2])
                # transpose xn -> eT[:, :, n*80 : n*80+80] (DMA xbar transpose)
                for cc in range(CEC):
                    nc.sync.dma_start(eT[:, cc, n * 80:(n + 1) * 80],
                                      xn[:, ts(cc, 128)], transpose=True)

                # ---------- kv projection ----------
                k_sb = sp.tile([128, CDC, S], BF16, tag="k_sb")
                for oc in range(CDC):
                    k_ps = psS.tile([128, S], F32, tag="sm")
                    for kc in range(CEC):
                        nc.tensor.matmul(
                            k_ps[:], kv_wT[:, kc, ts(oc, 128)],
                            eT[:, kc, n * 80:n * 80 + S],
                            start=(kc == 0), stop=(kc == CEC - 1))
                    nc.vector.tensor_scalar(
                        k_sb[:, oc, :], k_ps[:], scalar1=kv_b_k[:, oc:oc + 1],
                        scalar2=None, op0=ALU.add)
                v_ps = psS.tile([S, C], F32, tag="sm")
                for kc in range(CEC):
                    nc.tensor.matmul(
                        v_ps[:], eT[:, kc, n * 80:n * 80 + S],
                        kv_wT[:, kc, C:2 * C],
                        start=(kc == 0), stop=False)
                nc.tensor.matmul(v_ps[:], ones77[:], kv_b_v[:],
                                 start=False, stop=True)
                v_sb = sp.tile([S, NH * (D + 1)], BF16, tag="v_sb")
                for h in range(NH):
                    nc.vector.tensor_copy(
                        v_sb[:, h * (D + 1):h * (D + 1) + D],
                        v_ps[:, ts(h, D)])
                    nc.vector.memset(
                        v_sb[:, h * (D + 1) + D:(h + 1) * (D + 1)], 1.0)

                # ---------- attention per head ----------
                den_sb = sp.tile([NH, HW], BF16, tag="den_sb")
                y65s = []
                for h in range(NH):
                    pb = (h % 2) * D
                    oc = h // 2
                    att_ps = psA.tile([S, HW], F32, tag="big2")
                    for i in range(2):
                        nc.tensor.matmul(
                            att_ps[:, ts(i, 512)],
                            k_sb[pb:pb + D, oc, :],
                            q_bf[pb:pb + D, oc, ts(i, 512)],
                            start=True, stop=True)
                    atte = attp.tile([S, HW], BF16, tag="atte")
                    nc.scalar.activation(atte[:], att_ps[:], ACTF.Exp,
                                         bias=maskb[:, n:n + 1])
                    y_ps = psY.tile([D + 1, HW], F32, tag="y_ps")
                    for i in range(2):
                        nc.tensor.matmul(
                            y_ps[:, ts(i, 512)],
                            v_sb[:, h * (D + 1):(h + 1) * (D + 1)],
                            atte[:, ts(i, 512)],
                            start=True, stop=True)
                    y65 = y65p.tile([D + 1, HW], BF16, tag="y65")
                    y65s.append(y65)
                    if h % 2 == 0:
                        nc.scalar.activation(y65[:], y_ps[:], ACTF.Copy)
                    else:
                        nc.vector.tensor_copy(y65[:], y_ps[:])
                    nc.sync.dma_start(den_sb[h:h + 1, :], y65[D:D + 1, :])

                # ---------- softmax normalization ----------
                recip_s = sp.tile([NH, HW], BF16, tag="recip_s")
                with nc.allow_low_precision(reason="softmax denom recip in bf16"):
                    nc.vector.reciprocal(recip_s[:], den_sb[:])
                recip_d = dramp.tile([NH, HW], BF16, tag="recip_d")
                nc.sync.dma_start(recip_d[:], recip_s[:])
                y_sb = xp_pool.tile([128, CDC, HW], BF16, tag="y_sb")
                for h in range(NH):
                    pb = (h % 2) * D
                    oc = h // 2
                    rbc = attp.tile([D, HW], BF16, tag="rbc")
                    row = recip_d[h]
                    src = bass.AP(row.tensor, row.offset, [[0, D], [1, HW]])
                    nc.sync.dma_start(rbc[:], src)
                    nc.gpsimd.tensor_mul(
                        y_sb[pb:pb + D, oc, :], y65s[h][0:D, :], rbc[:])

                # ---------- out projection + residual ----------
                for oc in range(CDC):
                    o_ps = psA.tile([128, HW], F32, tag="big2")
                    for kc in range(CDC):
                        for nh2 in range(2):
                            nc.tensor.matmul(
                                o_ps[:, ts(nh2, 512)],
                                out_wT[:, kc, ts(oc, 128)],
                                y_sb[:, kc, ts(nh2, 512)],
                                start=(kc == 0), stop=(kc == CDC - 1))
                    o_f32 = outp.tile([128, HW], F32, tag="o_f32")
                    nc.vector.tensor_add(o_f32[:], o_ps[:], x_t[:, oc, :])
                    nc.sync.dma_start(
                        out_d[n, ts(oc, 128), :], o_f32[:])

            if reps == 1:
                body()
            else:
                with tc.For_i(0, reps, 1):
                    body()

    nc.compile()
    return nc


def _prep_host_inputs(input, cond, enc_hidden, enc_padding_mask,
                      adagn_w, adagn_b, ln_w, ln_b,
                      q_w, q_b, kv_w, kv_b, out_w, out_b):
    """Fold + transpose weights on host; return per-core input maps."""
    bf = ml_dtypes.bfloat16
    f32 = np.float32

    def chunked_T(wT, kc, cout):
        # wT: [cin, cout] -> [128, kc, cout]
        return np.ascontiguousarray(
            wT.reshape(kc, 128, cout).transpose(1, 0, 2))

    def pcol(b, nch):
        return np.ascontiguousarray(b.reshape(nch, 128).T)

    input = np.asarray(input, f32).reshape(N, C, HW)
    cond = np.asarray(cond, f32)
    enc_hidden = np.asarray(enc_hidden, f32)
    mask = np.asarray(enc_padding_mask, f32)
    adagn_w = np.asarray(adagn_w, f32); adagn_b_ = np.asarray(adagn_b, f32)
    ln_w = np.asarray(ln_w, f32); ln_b = np.asarray(ln_b, f32)
    q_w = np.asarray(q_w, f32); q_b_ = np.asarray(q_b, f32)
    kv_w = np.asarray(kv_w, f32); kv_b_ = np.asarray(kv_b, f32)
    out_w = np.asarray(out_w, f32); out_b_ = np.asarray(out_b, f32)

    # fold LN affine into kv
    kv_w_f = kv_w * ln_w[None, :]
    kv_b_f = kv_b_ + kv_w @ ln_b
    # fold attention scale into q and k-half
    q_w_f = q_w * SCALE
    q_b_f = q_b_ * SCALE
    kv_w_f[:C] *= SCALE
    kv_b_f[:C] *= SCALE
    # fold out_b into v bias: out_w @ delta = out_b
    if np.any(out_b_ != 0):
        delta = np.linalg.lstsq(out_w.astype(np.float64),
                                out_b_.astype(np.float64), rcond=None)[0]
        kv_b_f[C:] += delta.astype(f32)

    shared = {
        "adagn_wT": chunked_T(adagn_w.T, CDC, 2 * C).astype(bf),
        "adagn_b": pcol(adagn_b_, 2 * CDC),
        "q_wT": chunked_T(q_w_f.T, CDC, C).astype(bf),
        "q_b": pcol(q_b_f, CDC),
        "kv_wT": chunked_T(kv_w_f.T, CEC, 2 * C).astype(bf),
        "kv_b_k": pcol(kv_b_f[:C], CDC),
        "kv_b_v": kv_b_f[C:].astype(bf).reshape(1, C),
        "out_wT": chunked_T(out_w.T, CDC, C).astype(bf),
    }
    in_maps = []
    for core in range(N_CORES):
        sl = slice(core * NS, (core + 1) * NS)
        condT = np.ascontiguousarray(
            cond[sl].T.reshape(CDC, 128, NS).transpose(1, 0, 2)).astype(bf)
        m = dict(shared)
        m["x"] = np.ascontiguousarray(input[sl])
        m["enc"] = np.ascontiguousarray(enc_hidden[sl])
        m["condT"] = condT
        m["maskb"] = np.ascontiguousarray((mask[sl] * -10000.0).T)
        in_maps.append(m)
    return in_maps


_cached_nc = None


def kernel(**inputs) -> np.ndarray:
    global _cached_nc
    if _cached_nc is None:
        _cached_nc = build_program(reps=1)
    nc = _cached_nc
    in_maps = _prep_host_inputs(**inputs)
    res = run_bass_kernel_spmd(nc, in_maps, list(range(N_CORES)))
    out = np.concatenate([res.results[i]["out"] for i in range(N_CORES)],
                         axis=0)
    return out.reshape(N, C, H, W).astype(np.float32)
